# revision 37
# baseline (speedup 1.0000x reference)
"""Trainium2 Bass kernel for AliasFreeSampling.

Reference op per (b, c) plane X (512x512):
  reflect-pad 32 -> 65-tap separable lowpass -> 2x2 average pool -> Y (256x256)

The whole per-plane operator is linear and separable, so it folds into a
single 512x256 matrix D (pad + conv + pool combined):  Y = D^T @ X @ D.

On the PE array (out = lhsT.T @ rhs, contraction over partitions):
  phase 1: U^T = X^T @ D    via lhsT = X-chunk   [K=i,128][M=w,128],
                                 rhs = D-chunk   [K=i,128][N=j-window]
           -> U^T [w, j] comes out directly, no transposes anywhere.
  phase 2: Y   = U @ D      via lhsT = U^T-chunk [K=w,128][M=j,128],
                                 rhs = D-chunk   [K=w,128][N=c-window]

D is banded (65-tap filter + 2x pool stays local), so a contiguous 128-row
chunk of D only touches a ~96-wide window of the 256 output columns. Each
matmul therefore streams only its chunk's window instead of all 256 columns
(2.9x fewer PE cycles). PSUM accumulation handles the overlapping windows
for free: start=True on the first matmul marks the whole 2 KiB bank
pending-zero, later start=False matmuls add onto zeros (fresh columns) or
partials (overlap columns).

Sharding: pure data parallel - 256 (b,c) planes split as 32 planes on each
of the 8 NeuronCores; D is replicated; no cross-core communication.
"""

import numpy as np

import concourse.bacc as bacc
import concourse.bass as bass
import concourse.mybir as mybir
import concourse.tile as tile
from concourse.bass_utils import run_bass_kernel_spmd

N_CORES = 8
N_PLANES = 32        # planes per core
GROUP = 4            # planes per x-load / y-store DMA batch
H = W = 512
HO = WO = 256
PAD = 32
TAPS = 65

# matmul dtype mode: "f16"/"bf16" (cast inputs to 16-bit, 1 cycle/row on PE),
# "f32r" (reduced-precision single-pass, broken on HW), "f32" (full, 4x slower)
MM_MODE = "f16"

_MM16 = {"f16": mybir.dt.float16, "bf16": mybir.dt.bfloat16}


def _make_D(k: np.ndarray) -> np.ndarray:
    """Fold reflect-pad(32) + 65-tap conv + 2x avg-pool into one 512x256 map."""
    assert k.shape == (TAPS,)
    D = np.zeros((H, HO), dtype=np.float64)
    t = np.arange(TAPS)
    for j in range(HO):
        for r in (2 * j, 2 * j + 1):
            q = r + t - PAD
            i = np.where(q < 0, -q, np.where(q >= H, 2 * H - 2 - q, q))
            np.add.at(D[:, j], i, 0.5 * k.astype(np.float64))
    return D.astype(np.float32)


def _chunk_windows():
    """Per contiguous 128-row chunk of D, the column support window.

    Computed with all-ones taps: a superset of the true support for any tap
    values, so matmuls streaming just the window are exact. Consecutive
    windows overlap; union covers all 256 columns.
    """
    Dp = _make_D(np.ones(TAPS, dtype=np.float32))
    wins = []
    for c in range(4):
        nz = np.nonzero(np.any(Dp[c * 128:(c + 1) * 128] != 0.0, axis=0))[0]
        j0 = int(nz.min()) & ~1
        j1 = min(HO, (int(nz.max()) + 2) & ~1)
        wins.append((j0, j1))
    for a, b in zip(wins, wins[1:]):
        assert b[0] < a[1], f"windows must overlap for ordering: {wins}"
    return wins


def _emit(tc, y, x, d, n_planes, mode):
    nc = tc.nc
    f32 = mybir.dt.float32
    mm_cast = (lambda ap: ap.bitcast(mybir.dt.float32r)) if mode == "f32r" else (lambda ap: ap)
    WIN = _chunk_windows()
    n_groups = n_planes // GROUP

    from contextlib import ExitStack
    with ExitStack() as ctx:
        xpool = ctx.enter_context(tc.tile_pool(name="xin", bufs=3))
        dpool = ctx.enter_context(tc.tile_pool(name="dconst", bufs=1))
        utpool = ctx.enter_context(tc.tile_pool(name="ut", bufs=4))
        ypool = ctx.enter_context(tc.tile_pool(name="yout", bufs=3))
        pspool = ctx.enter_context(tc.tile_pool(name="ps", bufs=1, space="PSUM"))
        # D in natural 128-row chunks: d_sb[p, c, j] = D[128c+p, j]; serves as
        # the moving operand for both phases. Loaded via SWDGE (gpsimd): its
        # own queue, so it delays neither the x stream (sync ring) nor the
        # casts (scalar engine).
        d_sb = dpool.tile([128, 4, HO], d.dtype)
        dv = d.rearrange("(c p) j -> p c j", p=128)
        # per-chunk loads, c=0 first: the very first matmul only needs chunk 0,
        # so it isn't gated on the full 256 KiB D transfer at cold-DMA rates
        for c in range(4):
            nc.gpsimd.dma_start(d_sb[:, c], dv[:, c])

        ut_dt = _MM16.get(mode, f32)
        x_dt = _MM16.get(mode, f32)

        for g in range(n_groups):
            # x loads batched GROUP planes per dma_start on the sync ring:
            # 16 KiB contiguous per partition -> big descriptors, deep
            # rings (the 48-issue version starved the SDMA engines and ran
            # at 253 GB/s). bufs=6 paces the stream ~6 groups ahead of
            # compute; issuing ALL loads upfront (bufs=8) made the stream
            # finish by 57 us but stretched compute 25 us past it
            # (scheduler effect, measured +7 us total).
            xg = xpool.tile([128, GROUP, 4, W], x_dt, tag="x", bufs=6)
            if g == 0:
                # head: two half-group loads so the first matmuls start
                # after ~1 MiB instead of 2 MiB, with only 2 issue slots
                nc.sync.dma_start(xg[:, 0:2], x[g, :, 0:2])
                nc.sync.dma_start(xg[:, 2:4], x[g, :, 2:4])
            elif g == n_groups - 1:
                # tail: per-plane loads so the last plane's compute starts
                # as soon as its slice lands, not after the full 2 MiB; the
                # final plane lands in two c-pair halves so its first 8
                # matmuls overlap the last 256 KiB of the stream
                for pl in range(GROUP - 1):
                    nc.sync.dma_start(xg[:, pl], x[g, :, pl])
                nc.sync.dma_start(xg[:, GROUP - 1, 0:2], x[g, :, GROUP - 1, 0:2])
                nc.sync.dma_start(xg[:, GROUP - 1, 2:4], x[g, :, GROUP - 1, 2:4])
            else:
                nc.sync.dma_start(xg[:], x[g])

            y_sb = ypool.tile([128, GROUP, 2, WO], y.dtype, tag="y", bufs=3)
            for pl in range(GROUP):
                xmm = xg[:, pl]
                # all four wc's in ONE 2-bank PSUM tile: per-bank clear via
                # start=True on each bank's first matmul, then a single
                # whole-ut cast — casts have ~250ns fixed cost and the cast
                # engines are the straggler, so fewer/bigger casts win.
                ut = utpool.tile([128, 4, HO], ut_dt, tag="ut")
                ut_ps = pspool.tile([128, 4, HO], f32, tag="utps", bufs=3)
                # the very last plane iterates c-pair-major so matmuls on
                # the first half start while the second half still loads;
                # start/stop flags fire at the same (wc, c) in both orders
                if g == n_groups - 1 and pl == GROUP - 1:
                    mm_order = [(wc, c) for cp in range(2) for wc in range(4)
                                for c in (2 * cp, 2 * cp + 1)]
                else:
                    mm_order = [(wc, c) for wc in range(4) for c in range(4)]
                for wc, c in mm_order:
                    j0, j1 = WIN[c]
                    nc.tensor.matmul(
                        ut_ps[:, wc, j0:j1],
                        mm_cast(xmm[:, c, wc * 128:(wc + 1) * 128]),
                        mm_cast(d_sb[:, c, j0:j1]),
                        start=(wc % 2 == 0 and c == 0),
                        stop=(wc == 3 and c == 3),
                    )
                # casts alternate vector/scalar so no single engine
                # serializes the pipeline (gpsimd can't read PSUM)
                if pl % 2 == 0:
                    nc.vector.tensor_copy(ut[:], ut_ps[:])
                else:
                    nc.scalar.copy(ut[:], ut_ps[:])

                # both rr halves accumulate in ONE PSUM bank [128, 2, 256]
                # (rr0 -> cols 0:256, rr1 -> 256:512): start=True only on the
                # first matmul (bank-clear covers the whole 2 KiB bank), and a
                # single whole-bank cast replaces two half-bank ones
                # (casts have ~250ns fixed cost).
                utv = ut[:].rearrange("q wc (j2 rr) -> q wc j2 rr", rr=2)
                y_ps = pspool.tile([128, 2, WO], f32, tag="yps", bufs=2)
                for rr in range(2):
                    for wc in range(4):
                        j0, j1 = WIN[wc]
                        nc.tensor.matmul(
                            y_ps[:, rr, j0:j1],
                            mm_cast(utv[:, wc, :, rr]),
                            mm_cast(d_sb[:, wc, j0:j1]),
                            start=(rr == 0 and wc == 0),
                            stop=(rr == 1 and wc == 3),
                        )
                if pl % 2 == 1:
                    nc.vector.tensor_copy(y_sb[:, pl, :, :], y_ps[:])
                else:
                    nc.scalar.copy(y_sb[:, pl, :, :], y_ps[:])

                # Last group's store is split (planes 0-1, then 2, then 3),
                # emitted inline, so earlier halves drain while later
                # planes still compute and the final drain is 128 KiB.
                if g == n_groups - 1:
                    if pl == 1:
                        nc.scalar.dma_start(y[:, g, 0:2], y_sb[:, 0:2])
                    elif pl >= 2:
                        nc.scalar.dma_start(
                            y[:, g, pl:pl + 1], y_sb[:, pl:pl + 1])

            # y store batched GROUP planes: 4 KiB contiguous per partition.
            # On the scalar ring: NOT sync — a store issue waits for its
            # group's compute, and on the x ring it would block every x
            # issue queued behind it (measured: +24 us).
            if g != n_groups - 1:
                nc.scalar.dma_start(y[:, g], y_sb[:])


def build_nc(n_planes=N_PLANES, mode=MM_MODE):
    nc = bacc.Bacc("TRN2", target_bir_lowering=False, debug=False)
    f32 = mybir.dt.float32
    d_dt = _MM16.get(mode, f32)
    x_dt = _MM16.get(mode, f32)
    # x pre-permuted on host to [g, q, pl, c, w] (g = 4-plane group, q =
    # row-within-chunk, pl = plane-in-group, c = 128-row chunk): each SBUF
    # partition's 16 KiB group slice loads contiguously.
    x = nc.dram_tensor(
        "x", [n_planes // GROUP, 128, GROUP, 4, W], x_dt, kind="ExternalInput"
    ).ap()
    d = nc.dram_tensor("d", [H, HO], d_dt, kind="ExternalInput").ap()
    y_dt = _MM16.get(mode, f32)
    # y laid out [q, g, pl, rr, c] (output row = 2q + rr): per-partition
    # contiguous 4 KiB stores; host transposes back (free, untimed).
    y = nc.dram_tensor(
        "y", [128, n_planes // GROUP, GROUP, 2, WO], y_dt, kind="ExternalOutput"
    ).ap()
    with tile.TileContext(nc) as tc:
        _emit(tc, y, x, d, n_planes, mode)
    nc.compile()
    return nc


_NC_CACHE = {}


def _get_nc(n_planes=N_PLANES, mode=MM_MODE):
    key = (n_planes, mode)
    if key not in _NC_CACHE:
        _NC_CACHE[key] = build_nc(n_planes, mode)
    return _NC_CACHE[key]


def _d_input(k: np.ndarray, mode: str) -> np.ndarray:
    D = _make_D(k)
    if mode == "f16":
        return D.astype(np.float16)
    if mode == "bf16":
        import ml_dtypes
        return D.astype(ml_dtypes.bfloat16)
    return D


def kernel(x, kernel, **run_kwargs):
    x = np.asarray(x, dtype=np.float32)
    k = np.asarray(kernel, dtype=np.float32)
    B, C = x.shape[0], x.shape[1]
    assert x.shape == (B, C, H, W) and B * C == N_CORES * N_PLANES

    nc = _get_nc()
    d_in = _d_input(k, MM_MODE)
    if MM_MODE == "f16":
        x = x.astype(np.float16)
    elif MM_MODE == "bf16":
        import ml_dtypes
        x = x.astype(ml_dtypes.bfloat16)
    # [plane, h, w] -> [g, q, pl, c, w]: DRAM layout whose per-partition
    # reads are 16 KiB contiguous per group (see build_nc)
    xs = (
        x.reshape(N_CORES, N_PLANES // GROUP, GROUP, 4, 128, W)
        .transpose(0, 1, 4, 2, 3, 5)
    )
    in_maps = [
        {"x": np.ascontiguousarray(xs[c]), "d": d_in}
        for c in range(N_CORES)
    ]
    res = run_bass_kernel_spmd(nc, in_maps, core_ids=list(range(N_CORES)), **run_kwargs)
    # y arrives [q, g, pl, rr, c]; output row = 2q + rr
    y = np.stack([np.asarray(r["y"], dtype=np.float32) for r in res.results])
    y = y.transpose(0, 2, 3, 1, 4, 5)  # [core, g, pl, q, rr, c]
    out = y.reshape(B, C, HO, WO)
    if run_kwargs:
        return out, res
    return out



# revision 38
# speedup vs baseline: 1.0184x; 1.0184x over previous
"""Trainium2 Bass kernel for AliasFreeSampling.

Reference op per (b, c) plane X (512x512):
  reflect-pad 32 -> 65-tap separable lowpass -> 2x2 average pool -> Y (256x256)

The whole per-plane operator is linear and separable, so it folds into a
single 512x256 matrix D (pad + conv + pool combined):  Y = D^T @ X @ D.

On the PE array (out = lhsT.T @ rhs, contraction over partitions):
  phase 1: U^T = X^T @ D    via lhsT = X-chunk   [K=i,128][M=w,128],
                                 rhs = D-chunk   [K=i,128][N=j-window]
           -> U^T [w, j] comes out directly, no transposes anywhere.
  phase 2: Y   = U @ D      via lhsT = U^T-chunk [K=w,128][M=j,128],
                                 rhs = D-chunk   [K=w,128][N=c-window]

D is banded (65-tap filter + 2x pool stays local), so a contiguous 128-row
chunk of D only touches a ~96-wide window of the 256 output columns. Each
matmul therefore streams only its chunk's window instead of all 256 columns
(2.9x fewer PE cycles). PSUM accumulation handles the overlapping windows
for free: start=True on the first matmul marks the whole 2 KiB bank
pending-zero, later start=False matmuls add onto zeros (fresh columns) or
partials (overlap columns).

Sharding: pure data parallel - 256 (b,c) planes split as 32 planes on each
of the 8 NeuronCores; D is replicated; no cross-core communication.
"""

import numpy as np

import concourse.bacc as bacc
import concourse.bass as bass
import concourse.mybir as mybir
import concourse.tile as tile
from concourse.bass_utils import run_bass_kernel_spmd

N_CORES = 8
N_PLANES = 32        # planes per core
GROUP = 4            # planes per x-load / y-store DMA batch
H = W = 512
HO = WO = 256
PAD = 32
TAPS = 65

# matmul dtype mode: "f16"/"bf16" (cast inputs to 16-bit, 1 cycle/row on PE),
# "f32r" (reduced-precision single-pass, broken on HW), "f32" (full, 4x slower)
MM_MODE = "f16"

_MM16 = {"f16": mybir.dt.float16, "bf16": mybir.dt.bfloat16}


def _make_D(k: np.ndarray) -> np.ndarray:
    """Fold reflect-pad(32) + 65-tap conv + 2x avg-pool into one 512x256 map."""
    assert k.shape == (TAPS,)
    D = np.zeros((H, HO), dtype=np.float64)
    t = np.arange(TAPS)
    for j in range(HO):
        for r in (2 * j, 2 * j + 1):
            q = r + t - PAD
            i = np.where(q < 0, -q, np.where(q >= H, 2 * H - 2 - q, q))
            np.add.at(D[:, j], i, 0.5 * k.astype(np.float64))
    return D.astype(np.float32)


def _chunk_windows():
    """Per contiguous 128-row chunk of D, the column support window.

    Computed with all-ones taps: a superset of the true support for any tap
    values, so matmuls streaming just the window are exact. Consecutive
    windows overlap; union covers all 256 columns.
    """
    Dp = _make_D(np.ones(TAPS, dtype=np.float32))
    wins = []
    for c in range(4):
        nz = np.nonzero(np.any(Dp[c * 128:(c + 1) * 128] != 0.0, axis=0))[0]
        j0 = int(nz.min()) & ~1
        j1 = min(HO, (int(nz.max()) + 2) & ~1)
        wins.append((j0, j1))
    for a, b in zip(wins, wins[1:]):
        assert b[0] < a[1], f"windows must overlap for ordering: {wins}"
    return wins


def _emit(tc, y, x, d, n_planes, mode):
    nc = tc.nc
    f32 = mybir.dt.float32
    mm_cast = (lambda ap: ap.bitcast(mybir.dt.float32r)) if mode == "f32r" else (lambda ap: ap)
    WIN = _chunk_windows()
    n_groups = n_planes // GROUP

    from contextlib import ExitStack
    with ExitStack() as ctx:
        xpool = ctx.enter_context(tc.tile_pool(name="xin", bufs=3))
        dpool = ctx.enter_context(tc.tile_pool(name="dconst", bufs=1))
        utpool = ctx.enter_context(tc.tile_pool(name="ut", bufs=4))
        ypool = ctx.enter_context(tc.tile_pool(name="yout", bufs=3))
        pspool = ctx.enter_context(tc.tile_pool(name="ps", bufs=1, space="PSUM"))
        # D in natural 128-row chunks: d_sb[p, c, j] = D[128c+p, j]; serves as
        # the moving operand for both phases. Loaded via SWDGE (gpsimd): its
        # own queue, so it delays neither the x stream (sync ring) nor the
        # casts (scalar engine).
        d_sb = dpool.tile([128, 4, HO], d.dtype)
        dv = d.rearrange("(c p) j -> p c j", p=128)
        # per-chunk loads, c=0 first: the very first matmul only needs chunk 0,
        # so it isn't gated on the full 256 KiB D transfer at cold-DMA rates
        for c in range(4):
            nc.gpsimd.dma_start(d_sb[:, c], dv[:, c])

        ut_dt = _MM16.get(mode, f32)
        x_dt = _MM16.get(mode, f32)

        for g in range(n_groups):
            # x loads batched GROUP planes per dma_start on the sync ring:
            # 16 KiB contiguous per partition -> big descriptors, deep
            # rings (the 48-issue version starved the SDMA engines and ran
            # at 253 GB/s). bufs=6 paces the stream ~6 groups ahead of
            # compute; issuing ALL loads upfront (bufs=8) made the stream
            # finish by 57 us but stretched compute 25 us past it
            # (scheduler effect, measured +7 us total).
            xg = xpool.tile([128, GROUP, 4, W], x_dt, tag="x", bufs=6)
            if g == 0:
                # head: two half-group loads so the first matmuls start
                # after ~1 MiB instead of 2 MiB, with only 2 issue slots
                nc.sync.dma_start(xg[:, 0:2], x[g, :, 0:2])
                nc.sync.dma_start(xg[:, 2:4], x[g, :, 2:4])
            elif g == n_groups - 1:
                # tail: per-plane loads so the last plane's compute starts
                # as soon as its slice lands, not after the full 2 MiB; the
                # final plane lands in two c-pair halves so its first 8
                # matmuls overlap the last 256 KiB of the stream
                for pl in range(GROUP - 1):
                    nc.sync.dma_start(xg[:, pl], x[g, :, pl])
                nc.sync.dma_start(xg[:, GROUP - 1, 0:2], x[g, :, GROUP - 1, 0:2])
                nc.sync.dma_start(xg[:, GROUP - 1, 2:4], x[g, :, GROUP - 1, 2:4])
            else:
                nc.sync.dma_start(xg[:], x[g])

            y_sb = ypool.tile([128, GROUP, 2, WO], y.dtype, tag="y", bufs=3)
            for pl in range(GROUP):
                xmm = xg[:, pl]
                # all four wc's in ONE 2-bank PSUM tile: per-bank clear via
                # start=True on each bank's first matmul, then a single
                # whole-ut cast — casts have ~250ns fixed cost and the cast
                # engines are the straggler, so fewer/bigger casts win.
                ut = utpool.tile([128, 4, HO], ut_dt, tag="ut")
                ut_ps = pspool.tile([128, 4, HO], f32, tag="utps", bufs=2)
                # the very last plane iterates c-pair-major so matmuls on
                # the first half start while the second half still loads;
                # start/stop flags fire at the same (wc, c) in both orders
                if g == n_groups - 1 and pl == GROUP - 1:
                    mm_order = [(wc, c) for cp in range(2) for wc in range(4)
                                for c in (2 * cp, 2 * cp + 1)]
                else:
                    mm_order = [(wc, c) for wc in range(4) for c in range(4)]
                for wc, c in mm_order:
                    j0, j1 = WIN[c]
                    nc.tensor.matmul(
                        ut_ps[:, wc, j0:j1],
                        mm_cast(xmm[:, c, wc * 128:(wc + 1) * 128]),
                        mm_cast(d_sb[:, c, j0:j1]),
                        start=(wc % 2 == 0 and c == 0),
                        stop=(wc == 3 and c == 3),
                    )
                # casts alternate vector/scalar so no single engine
                # serializes the pipeline (gpsimd can't read PSUM)
                if pl % 2 == 0:
                    nc.vector.tensor_copy(ut[:], ut_ps[:])
                else:
                    nc.scalar.copy(ut[:], ut_ps[:])

                # both rr halves accumulate in ONE PSUM bank [128, 2, 256]
                # (rr0 -> cols 0:256, rr1 -> 256:512): start=True only on the
                # first matmul (bank-clear covers the whole 2 KiB bank), and a
                # single whole-bank cast replaces two half-bank ones
                # (casts have ~250ns fixed cost).
                utv = ut[:].rearrange("q wc (j2 rr) -> q wc j2 rr", rr=2)
                y_ps = pspool.tile([128, 2, WO], f32, tag="yps", bufs=3)
                for rr in range(2):
                    for wc in range(4):
                        j0, j1 = WIN[wc]
                        nc.tensor.matmul(
                            y_ps[:, rr, j0:j1],
                            mm_cast(utv[:, wc, :, rr]),
                            mm_cast(d_sb[:, wc, j0:j1]),
                            start=(rr == 0 and wc == 0),
                            stop=(rr == 1 and wc == 3),
                        )
                if pl % 2 == 1:
                    nc.vector.tensor_copy(y_sb[:, pl, :, :], y_ps[:])
                else:
                    nc.scalar.copy(y_sb[:, pl, :, :], y_ps[:])

                # Last group's store is split (planes 0-1, then 2, then 3),
                # emitted inline, so earlier halves drain while later
                # planes still compute and the final drain is 128 KiB.
                if g == n_groups - 1:
                    if pl == 1:
                        nc.scalar.dma_start(y[:, g, 0:2], y_sb[:, 0:2])
                    elif pl >= 2:
                        nc.scalar.dma_start(
                            y[:, g, pl:pl + 1], y_sb[:, pl:pl + 1])

            # y store batched GROUP planes: 4 KiB contiguous per partition.
            # On the scalar ring: NOT sync — a store issue waits for its
            # group's compute, and on the x ring it would block every x
            # issue queued behind it (measured: +24 us).
            if g != n_groups - 1:
                nc.scalar.dma_start(y[:, g], y_sb[:])


def build_nc(n_planes=N_PLANES, mode=MM_MODE):
    nc = bacc.Bacc("TRN2", target_bir_lowering=False, debug=False)
    f32 = mybir.dt.float32
    d_dt = _MM16.get(mode, f32)
    x_dt = _MM16.get(mode, f32)
    # x pre-permuted on host to [g, q, pl, c, w] (g = 4-plane group, q =
    # row-within-chunk, pl = plane-in-group, c = 128-row chunk): each SBUF
    # partition's 16 KiB group slice loads contiguously.
    x = nc.dram_tensor(
        "x", [n_planes // GROUP, 128, GROUP, 4, W], x_dt, kind="ExternalInput"
    ).ap()
    d = nc.dram_tensor("d", [H, HO], d_dt, kind="ExternalInput").ap()
    y_dt = _MM16.get(mode, f32)
    # y laid out [q, g, pl, rr, c] (output row = 2q + rr): per-partition
    # contiguous 4 KiB stores; host transposes back (free, untimed).
    y = nc.dram_tensor(
        "y", [128, n_planes // GROUP, GROUP, 2, WO], y_dt, kind="ExternalOutput"
    ).ap()
    with tile.TileContext(nc) as tc:
        _emit(tc, y, x, d, n_planes, mode)
    nc.compile()
    return nc


_NC_CACHE = {}


def _get_nc(n_planes=N_PLANES, mode=MM_MODE):
    key = (n_planes, mode)
    if key not in _NC_CACHE:
        _NC_CACHE[key] = build_nc(n_planes, mode)
    return _NC_CACHE[key]


def _d_input(k: np.ndarray, mode: str) -> np.ndarray:
    D = _make_D(k)
    if mode == "f16":
        return D.astype(np.float16)
    if mode == "bf16":
        import ml_dtypes
        return D.astype(ml_dtypes.bfloat16)
    return D


def kernel(x, kernel, **run_kwargs):
    x = np.asarray(x, dtype=np.float32)
    k = np.asarray(kernel, dtype=np.float32)
    B, C = x.shape[0], x.shape[1]
    assert x.shape == (B, C, H, W) and B * C == N_CORES * N_PLANES

    nc = _get_nc()
    d_in = _d_input(k, MM_MODE)
    if MM_MODE == "f16":
        x = x.astype(np.float16)
    elif MM_MODE == "bf16":
        import ml_dtypes
        x = x.astype(ml_dtypes.bfloat16)
    # [plane, h, w] -> [g, q, pl, c, w]: DRAM layout whose per-partition
    # reads are 16 KiB contiguous per group (see build_nc)
    xs = (
        x.reshape(N_CORES, N_PLANES // GROUP, GROUP, 4, 128, W)
        .transpose(0, 1, 4, 2, 3, 5)
    )
    in_maps = [
        {"x": np.ascontiguousarray(xs[c]), "d": d_in}
        for c in range(N_CORES)
    ]
    res = run_bass_kernel_spmd(nc, in_maps, core_ids=list(range(N_CORES)), **run_kwargs)
    # y arrives [q, g, pl, rr, c]; output row = 2q + rr
    y = np.stack([np.asarray(r["y"], dtype=np.float32) for r in res.results])
    y = y.transpose(0, 2, 3, 1, 4, 5)  # [core, g, pl, q, rr, c]
    out = y.reshape(B, C, HO, WO)
    if run_kwargs:
        return out, res
    return out

